# revision 7
# baseline (speedup 1.0000x reference)
"""Trainium2 Bass kernel for nn_KVCacheAttention (GQA attention prefill layer).

Reference computation (B=2, S=2048, HID=2048, H=16, KVH=4, D=128):
    q = x @ Wq.T ; k = x @ Wk.T ; v = x @ Wv.T          (per-head, RoPE on q,k)
    scores = (q @ k.T) * 1/sqrt(D) + causal_mask
    out    = softmax(scores) @ v
    y      = out @ Wo.T

Sharding: tensor-parallel over heads across 8 cores. Core c owns q-heads
{2c, 2c+1} and kv-head c//2, plus the matching 256 columns of Wo. Each core
produces a partial y[4096, 2048]; the host sums the 8 partials.

Device layout notes:
  - All matmuls run as float32r (fp32 bits, full 1 col/cycle PE rate for
    moving dim >= 256).
  - Host pre-transposes x -> xT [HID, B*S] and weights so every DMA is
    contiguous and the contraction dim always lands on partitions.
  - Softmax is row-wise two-pass (exact row max), with the exp's row-sum
    produced for free via the activation accum_out port. Probabilities are
    normalized in-place, then PE-transposed per 128x128 tile for the PV
    matmul (contraction over key positions).
  - Causality: only lower-triangle 128x128 score blocks are computed; the
    diagonal block gets an additive mask constant. The host verifies the
    provided attn_mask actually is the causal mask and falls back to a
    numpy path if not.
"""

import numpy as np

B, S, HID = 2, 2048, 2048
H, KVH, D = 16, 4, 128
GROUPS = H // KVH
THETA = 10000.0
SCALE = 1.0 / float(np.sqrt(D).astype(np.float32))
NEG = -1e9
NCORES = 8
BS = B * S

_cached = {}


def _build_nc(s, hid, nqh=2):
    """Build the per-core Bass program. s=seq len, hid=model dim, nqh=#q heads.

    DRAM tensors (per core):
      xT   [hid, B*s]    input, transposed
      wqT  [hid, nqh*D]  q weights, transposed slice
      wkT  [hid, D]      k weights
      wvT  [hid, D]      v weights
      woT  [nqh*D, hid]  o weights slice (rows = this core's out-proj inputs)
      cs   [2, 128, s]   per batch: rows 0:64 cos table, 64:128 sin table
      mblk [128, 128]    additive causal mask for diagonal blocks (pre-divided
                         by SCALE)
      y    [B*s, hid]    partial output
    """
    import concourse.bass as bass
    import concourse.bacc as bacc
    import concourse.tile as tile
    from concourse import mybir
    from concourse.masks import make_identity
    from contextlib import ExitStack

    f32 = mybir.dt.float32
    f32r = mybir.dt.float32r
    AX = mybir.AxisListType.X
    Exp = mybir.ActivationFunctionType.Exp

    bs = B * s
    SK = hid // 128          # contraction tiles for projections
    NPC = bs // 512          # 512-wide position chunks (projection loop)
    NB = s // 128            # 128-row q-blocks per batch
    NW = s // 512            # 512-wide attention windows per batch
    NHC = hid // 512         # 512-wide hid chunks (out-proj loop)

    nc = bacc.Bacc(None, target_bir_lowering=False, debug=False)
    xT = nc.dram_tensor("xT", [hid, bs], f32, kind="ExternalInput")
    wqT = nc.dram_tensor("wqT", [hid, nqh * D], f32, kind="ExternalInput")
    wkT = nc.dram_tensor("wkT", [hid, D], f32, kind="ExternalInput")
    wvT = nc.dram_tensor("wvT", [hid, D], f32, kind="ExternalInput")
    woT = nc.dram_tensor("woT", [nqh * D, hid], f32, kind="ExternalInput")
    cs = nc.dram_tensor("cs", [B, 128, s], f32, kind="ExternalInput")
    mblk = nc.dram_tensor("mblk", [128, 128], f32, kind="ExternalInput")
    y = nc.dram_tensor("y", [bs, hid], f32, kind="ExternalOutput")

    def r(ap):
        return ap.bitcast(f32r)

    with tile.TileContext(nc) as tc, ExitStack() as octx:
        outer = octx.enter_context(tc.tile_pool(name="outer", bufs=1))
        ident = outer.tile([128, 128], f32)
        make_identity(nc, ident)
        mb = outer.tile([128, 128], f32)
        nc.sync.dma_start(mb[:], mblk[:])

        # resident q/k/v activations, [128, ...] with d on partitions for q/k
        # (qk layout: [d, b*s]) and positions on partitions for v ([pos, d]
        # packed as [128, NB*B, 128]).
        qkv = octx.enter_context(tc.tile_pool(name="qkv", bufs=1))
        qsb = qkv.tile([128, nqh, bs], f32r)
        ksb = qkv.tile([128, bs], f32r)
        vsb = qkv.tile([128, B * NB, 128], f32r)

        # ---------------- phase 1: projections + rope -----------------
        with ExitStack() as ctx:
            wpool = ctx.enter_context(tc.tile_pool(name="wpool", bufs=1))
            wq = wpool.tile([128, SK, nqh * D], f32)
            wk = wpool.tile([128, SK, D], f32)
            wv = wpool.tile([128, SK, D], f32)
            cst = wpool.tile([128, B, s], f32)
            wqr = wpool.tile([128, SK, nqh * D], f32r)
            wkr = wpool.tile([128, SK, D], f32r)
            wvr = wpool.tile([128, SK, D], f32r)
            nc.sync.dma_start(wq[:], wqT.rearrange("(t p) m -> p t m", p=128))
            nc.sync.dma_start(wk[:], wkT.rearrange("(t p) m -> p t m", p=128))
            nc.sync.dma_start(wv[:], wvT.rearrange("(t p) m -> p t m", p=128))
            nc.sync.dma_start(cst[:], cs.rearrange("b p s -> p b s"))
            nc.vector.tensor_copy(wqr[:], wq[:])
            nc.vector.tensor_copy(wkr[:], wk[:])
            nc.vector.tensor_copy(wvr[:], wv[:])

            xpool = ctx.enter_context(tc.tile_pool(name="xpool", bufs=4))
            tpool = ctx.enter_context(tc.tile_pool(name="tpool", bufs=3))
            vtmp = ctx.enter_context(tc.tile_pool(name="vtmp", bufs=2))
            pps = ctx.enter_context(
                tc.tile_pool(name="pps", bufs=2, space="PSUM"))
            vtps = ctx.enter_context(
                tc.tile_pool(name="vtps", bufs=2, space="PSUM"))

            for c in range(NPC):
                bb = (c * 512) // s          # batch of this chunk
                sl0 = (c * 512) % s          # in-batch position offset
                qp = [pps.tile([128, 512], f32, tag="qp", name=f"qp{_h}")
                      for _h in range(nqh)]
                kp = pps.tile([128, 512], f32, tag="kp")
                vp = pps.tile([128, 512], f32, tag="vp")
                for kt in range(SK):
                    xt = xpool.tile([128, 512], f32, tag="xt")
                    nc.sync.dma_start(
                        xt[:], xT[kt * 128:(kt + 1) * 128, c * 512:(c + 1) * 512])
                    xr = xpool.tile([128, 512], f32r, tag="xr")
                    nc.vector.tensor_copy(xr[:], xt[:])
                    st, sp = (kt == 0), (kt == SK - 1)
                    for h in range(nqh):
                        nc.tensor.matmul(
                            qp[h][:], wqr[:, kt, h * D:(h + 1) * D], xr[:],
                            start=st, stop=sp)
                    nc.tensor.matmul(kp[:], wkr[:, kt, :], xr[:],
                                     start=st, stop=sp)
                    nc.tensor.matmul(vp[:], wvr[:, kt, :], xr[:],
                                     start=st, stop=sp)

                # rope: dst = src*cos + rot(src)*sin, cos/sin tables are
                # 64-row; rows 64:128 of cst hold sin.
                co = cst[0:64, bb, sl0:sl0 + 512]
                si = cst[64:128, bb, sl0:sl0 + 512]
                for h in range(nqh + 1):
                    src = qp[h] if h < nqh else kp
                    dst = (qsb[:, h, c * 512:(c + 1) * 512] if h < nqh
                           else ksb[:, c * 512:(c + 1) * 512])
                    tmp = tpool.tile([128, 512], f32, tag="tmp")
                    tmc = tpool.tile([128, 512], f32, tag="tmc")
                    nc.vector.tensor_mul(tmc[0:64, :], src[0:64, :], co)
                    nc.vector.tensor_mul(tmc[64:128, :], src[64:128, :], co)
                    nc.vector.tensor_mul(tmp[0:64, :], src[64:128, :], si)
                    nc.vector.tensor_mul(tmp[64:128, :], src[0:64, :], si)
                    nc.vector.tensor_sub(dst[0:64, :], tmc[0:64, :],
                                         tmp[0:64, :])
                    nc.vector.tensor_add(dst[64:128, :], tmc[64:128, :],
                                         tmp[64:128, :])

                # v: evict to sbuf, then PE-transpose to [pos, d] tiles
                vt = vtmp.tile([128, 512], f32)
                nc.scalar.copy(vt[:], vp[:])
                for j in range(4):
                    vtp = vtps.tile([128, 128], f32)
                    nc.tensor.transpose(vtp[:], vt[:, j * 128:(j + 1) * 128],
                                        ident[:])
                    nc.scalar.copy(vsb[:, c * 4 + j, :], vtp[:])

        # ---------------- phase 2: attention + out-proj ----------------
        with ExitStack() as ctx:
            wopool = ctx.enter_context(tc.tile_pool(name="wopool", bufs=1))
            wo = wopool.tile([128, nqh, hid], f32)
            wor = wopool.tile([128, nqh, hid], f32r)
            nc.sync.dma_start(wo[:], woT.rearrange("(t p) m -> p t m", p=128))
            nc.vector.tensor_copy(wor[:], wo[:])

            ppool = ctx.enter_context(tc.tile_pool(name="ppool", bufs=2))
            ptw = ctx.enter_context(tc.tile_pool(name="ptw", bufs=1))
            opool = ctx.enter_context(tc.tile_pool(name="opool", bufs=2))
            ybuf = ctx.enter_context(tc.tile_pool(name="ybuf", bufs=3))
            stat = ctx.enter_context(tc.tile_pool(name="stat", bufs=8))
            scps = ctx.enter_context(
                tc.tile_pool(name="scps", bufs=4, space="PSUM"))
            ptps = ctx.enter_context(
                tc.tile_pool(name="ptps", bufs=2, space="PSUM"))
            ops = ctx.enter_context(
                tc.tile_pool(name="ops", bufs=1, space="PSUM"))
            yps = ctx.enter_context(
                tc.tile_pool(name="yps", bufs=1, space="PSUM"))

            for b in range(B):
                for w in range(NW):
                    imax = w * 4 + 3       # last q-block in this window
                    osb = opool.tile([128, nqh, 512], f32r)
                    for h in range(nqh):
                        qh = qsb[:, h, b * s:(b + 1) * s]
                        kh = ksb[:, b * s:(b + 1) * s]
                        pts = [ptw.tile([128, 512], f32r, tag=f"pt{j}", name=f"pt{j}")
                               for j in range(imax + 1)]
                        for i in range(w * 4, w * 4 + 4):
                            kw = (i + 1) * 128           # causal key width
                            nch = (kw + 511) // 512
                            rmp = stat.tile([128, 8], f32, tag="rmp")
                            rsp = stat.tile([128, 8], f32, tag="rsp")
                            sps = []
                            for kc in range(nch):
                                cw = min(512, kw - kc * 512)
                                sp = scps.tile([128, 512], f32, tag="sc")
                                sps.append(sp)
                                nc.tensor.matmul(
                                    sp[:, :cw],
                                    qh[:, i * 128:(i + 1) * 128],
                                    kh[:, kc * 512:kc * 512 + cw],
                                    start=True, stop=True)
                                if kc == nch - 1:
                                    off = i * 128 - kc * 512
                                    nc.vector.tensor_add(
                                        sp[:, off:off + 128],
                                        sp[:, off:off + 128], mb[:])
                                nc.vector.reduce_max(
                                    rmp[:, kc:kc + 1], sp[:, :cw], axis=AX)
                            rm = stat.tile([128, 1], f32, tag="rm")
                            nb_ = stat.tile([128, 1], f32, tag="nb")
                            nc.vector.reduce_max(rm[:], rmp[:, :nch], axis=AX)
                            nc.vector.tensor_scalar_mul(nb_[:], rm[:], -SCALE)
                            pst = ppool.tile([128, s], f32)
                            for kc in range(nch):
                                cw = min(512, kw - kc * 512)
                                nc.scalar.activation(
                                    pst[:, kc * 512:kc * 512 + cw],
                                    sps[kc][:, :cw], Exp,
                                    bias=nb_[:, 0:1], scale=SCALE,
                                    accum_out=rsp[:, kc:kc + 1])
                            rs = stat.tile([128, 1], f32, tag="rs")
                            ri = stat.tile([128, 1], f32, tag="ri")
                            nc.vector.reduce_sum(rs[:], rsp[:, :nch], axis=AX)
                            nc.vector.reciprocal(ri[:], rs[:])
                            nc.vector.tensor_scalar_mul(
                                pst[:, :kw], pst[:, :kw], ri[:, 0:1])
                            # transpose each 128-block of p into window strips
                            iw = i - w * 4
                            for j in range(i + 1):
                                ptp = ptps.tile([128, 128], f32)
                                nc.tensor.transpose(
                                    ptp[:], pst[:, j * 128:(j + 1) * 128],
                                    ident[:])
                                nc.scalar.copy(
                                    pts[j][:, iw * 128:(iw + 1) * 128], ptp[:])
                        # PV: out_h^T[d, qwin] accumulated over key tiles
                        op = ops.tile([128, 512], f32)
                        for j in range(imax + 1):
                            off = max(0, j * 128 - w * 512)
                            nc.tensor.matmul(
                                op[:, off:512],
                                vsb[:, b * NB + j, :],
                                pts[j][:, off:512],
                                start=(j == 0), stop=(j == imax))
                        nc.scalar.copy(osb[:, h, :], op[:])
                    # out-proj for this (b, w): y[pos, hid] partial
                    for pb in range(4):
                        yb = ybuf.tile([128, hid], f32)
                        for hc in range(NHC):
                            yp = yps.tile([128, 512], f32)
                            for kt in range(nqh):
                                nc.tensor.matmul(
                                    yp[:],
                                    osb[:, kt, pb * 128:(pb + 1) * 128],
                                    wor[:, kt, hc * 512:(hc + 1) * 512],
                                    start=(kt == 0), stop=(kt == nqh - 1))
                            nc.scalar.copy(yb[:, hc * 512:(hc + 1) * 512],
                                           yp[:])
                        row = b * s + w * 512 + pb * 128
                        nc.sync.dma_start(y[row:row + 128, :], yb[:])
    nc.compile()
    return nc


def _prep_inputs(x, pos_ids, Wq, Wk, Wv, Wo, s, hid, nqh=2, ncores=NCORES):
    """Build the 8 per-core input maps (host-side shard + reformat)."""
    bs = B * s
    xT = np.ascontiguousarray(x.reshape(bs, hid).T)
    inv = (1.0 / (THETA ** (np.arange(0, D, 2, dtype=np.float32) / D))).astype(
        np.float32)
    cs = np.empty((B, 128, s), dtype=np.float32)
    for b in range(B):
        fr = pos_ids[b].astype(np.float32)[None, :] * inv[:, None]  # [64, s]
        cs[b, 0:64] = np.cos(fr)
        cs[b, 64:128] = np.sin(fr)
    mblk = np.triu(np.full((128, 128), NEG / SCALE, dtype=np.float32), k=1)
    kvh_d = KVH * D // (ncores // GROUPS) if False else None  # unused
    in_maps = []
    for c in range(ncores):
        h0 = c * nqh                       # first q head of this core
        g = h0 // GROUPS                   # kv head of this core
        wqT = np.ascontiguousarray(
            Wq[h0 * D:(h0 + nqh) * D, :].T)            # [hid, nqh*D]
        wkT = np.ascontiguousarray(Wk[g * D:(g + 1) * D, :].T)   # [hid, D]
        wvT = np.ascontiguousarray(Wv[g * D:(g + 1) * D, :].T)   # [hid, D]
        woT = np.ascontiguousarray(Wo[:, h0 * D:(h0 + nqh) * D].T)  # [nqh*D, hid]
        in_maps.append({"xT": xT, "wqT": wqT, "wkT": wkT, "wvT": wvT,
                        "woT": woT, "cs": cs, "mblk": mblk})
    return in_maps


def _numpy_fallback(x, attn_mask, pos_ids, Wq, Wk, Wv, Wo):
    b, s, _ = x.shape
    q = (x.reshape(b * s, HID) @ Wq.T).reshape(b, s, H, D).transpose(0, 2, 1, 3)
    k = (x.reshape(b * s, HID) @ Wk.T).reshape(b, s, KVH, D).transpose(0, 2, 1, 3)
    v = (x.reshape(b * s, HID) @ Wv.T).reshape(b, s, KVH, D).transpose(0, 2, 1, 3)
    inv = 1.0 / (THETA ** (np.arange(0, D, 2, dtype=np.float32) / D))
    fr = pos_ids.astype(np.float32)[:, :, None] * inv
    emb = np.concatenate([fr, fr], axis=-1)
    cos, sin = np.cos(emb)[:, None], np.sin(emb)[:, None]

    def rot(t):
        t1, t2 = np.split(t, 2, axis=-1)
        return np.concatenate([-t2, t1], axis=-1)

    q = q * cos + rot(q) * sin
    k = k * cos + rot(k) * sin
    k = np.repeat(k, GROUPS, axis=1)
    v = np.repeat(v, GROUPS, axis=1)
    out = np.empty((b, H, s, D), dtype=np.float32)
    for bi in range(b):
        for h in range(H):
            sc = (q[bi, h] @ k[bi, h].T) * SCALE + attn_mask[bi, 0]
            sc -= sc.max(axis=-1, keepdims=True)
            p = np.exp(sc)
            p /= p.sum(axis=-1, keepdims=True)
            out[bi, h] = p @ v[bi, h]
    out = out.transpose(0, 2, 1, 3).reshape(b, s, H * D)
    return (out.reshape(b * s, H * D) @ Wo.T).reshape(b, s, HID).astype(
        np.float32)


def _is_causal(attn_mask):
    if attn_mask.shape != (B, 1, S, S):
        return False
    causal = np.triu(np.full((S, S), NEG, dtype=np.float32), k=1)
    return all(np.array_equal(attn_mask[b, 0], causal) for b in range(B))


def kernel(x, attn_mask, pos_ids, Wq, Wk, Wv, Wo):
    x = np.asarray(x, dtype=np.float32)
    attn_mask = np.asarray(attn_mask, dtype=np.float32)
    pos_ids = np.asarray(pos_ids)
    Wq = np.asarray(Wq, dtype=np.float32)
    Wk = np.asarray(Wk, dtype=np.float32)
    Wv = np.asarray(Wv, dtype=np.float32)
    Wo = np.asarray(Wo, dtype=np.float32)

    if (x.shape != (B, S, HID) or not _is_causal(attn_mask)
            or Wq.shape != (H * D, HID) or Wk.shape != (KVH * D, HID)):
        return _numpy_fallback(x, attn_mask, pos_ids, Wq, Wk, Wv, Wo)

    from concourse.bass_utils import run_bass_kernel_spmd

    if "nc" not in _cached:
        _cached["nc"] = _build_nc(S, HID)
    in_maps = _prep_inputs(x, pos_ids, Wq, Wk, Wv, Wo, S, HID)
    res = run_bass_kernel_spmd(_cached["nc"], in_maps,
                               core_ids=list(range(NCORES)))
    y = res.results[0]["y"].astype(np.float64)
    for c in range(1, NCORES):
        y += res.results[c]["y"]
    return y.astype(np.float32).reshape(B, S, HID)
